# revision 11
# baseline (speedup 1.0000x reference)
"""Trainium2 Bass kernel for nn_CrossAttention (B=8, SQ=4096, SKV=77,
D_EMBED=1024, D_CROSS=768, H=8, Dh=128), fp32 in/out.

Strategy: data-parallel over batch — one batch element per NeuronCore (8
cores).  Per core, everything is computed in "transposed" space (features on
partitions) so that no operand ever needs a transpose except the input x
itself (done on the PE via is_transpose matmuls) and softmax sums (done as a
ones-matmul over the partition axis):

  xT  = transpose(x_blk)                      [1024, QB]   (PE transpose)
  qT  = Wq.T @ xT + bq                        [1024, QB]   (features on part.)
  kT  = Wk.T @ yT + bk                        [1024, 77]
  v   = y @ Wv + bv                           [77, 1024]
  sT_h  = kT_h.T @ qT_h  (K=Dh on part.)      [77, QB] per head
  e_h   = exp(sT_h / sqrt(Dh))                [77, QB]
  sum_h = ones.T @ e_h                        [1, QB]
  uT_h  = v_h.T @ e_h    (K=77 on part.)      [128, QB]
  aT_h  = uT_h * broadcast(1/sum_h)           [128, QB]
  out   = aT.T @ Wo + bo                      [QB, 1024]  (queries on part.)

Matmuls run in float32r (full-rate fp32 PE mode); operand tensors/tiles are
declared float32r end-to-end (numpy sees plain float32 — same bytes).
"""

import math
import os
from contextlib import ExitStack

import numpy as np

os.environ.setdefault("JAX_PLATFORMS", "cpu")

import concourse.bass as bass  # noqa: E402
import concourse.tile as tile  # noqa: E402
from concourse import bacc, mybir  # noqa: E402
from concourse import bass_utils  # noqa: E402
from concourse.masks import make_identity  # noqa: E402

F32 = mybir.dt.float32
F32R = mybir.dt.float32r
AF = mybir.ActivationFunctionType

# Persist compiled executables across processes: the bass_exec HLO is
# deterministic for a given build, so the (slow) neuronxcc compile is hit
# only once per machine.
try:
    import jax

    jax.config.update("jax_compilation_cache_dir", "/tmp/jax_bass_cache")
    jax.config.update("jax_persistent_cache_min_compile_time_secs", 0)
    jax.config.update("jax_persistent_cache_min_entry_size_bytes", 0)
except Exception:
    pass

# Problem constants (hardcoded per contract).
B = 8
SQ = 4096
SKV = 77
DE = 1024
DC = 768
H = 8
DH = 128
NCORES = 8

QB = 512          # queries per block
ME = DE // 128    # 8 feature chunks of the embed dim
MC = DC // 128    # 6 chunks of the cross dim
SCALE = 1.0 / math.sqrt(DH)
SKVP = 80          # SKV padded to even size (fp32r dst patterns must be even)


def build_nc(sq=SQ):
    """Build the per-core Bass program (identical on all cores)."""
    nblk = sq // QB
    nc = bacc.Bacc(
        "TRN2",
        target_bir_lowering=False,
        debug=False,
        enable_asserts=False,
        num_devices=NCORES,
    )
    x = nc.dram_tensor("x", [sq, DE], F32, kind="ExternalInput").ap()
    y = nc.dram_tensor("y", [SKV, DC], F32, kind="ExternalInput").ap()
    Wq = nc.dram_tensor("Wq", [DE, DE], F32R, kind="ExternalInput").ap()
    bq = nc.dram_tensor("bq", [DE], F32, kind="ExternalInput").ap()
    Wk = nc.dram_tensor("Wk", [DC, DE], F32R, kind="ExternalInput").ap()
    bk = nc.dram_tensor("bk", [DE], F32, kind="ExternalInput").ap()
    Wv = nc.dram_tensor("Wv", [DC, DE], F32R, kind="ExternalInput").ap()
    bv = nc.dram_tensor("bv", [DE], F32, kind="ExternalInput").ap()
    Wo = nc.dram_tensor("Wo", [DE, DE], F32R, kind="ExternalInput").ap()
    bo = nc.dram_tensor("bo", [DE], F32, kind="ExternalInput").ap()
    out = nc.dram_tensor("out", [sq, DE], F32, kind="ExternalOutput").ap()

    with tile.TileContext(nc) as tc, ExitStack() as ctx:
        const = ctx.enter_context(tc.tile_pool(name="const", bufs=1))
        wq_pool = ctx.enter_context(tc.tile_pool(name="wq", bufs=1))
        wo_pool = ctx.enter_context(tc.tile_pool(name="wo", bufs=1))
        kv_keep = ctx.enter_context(tc.tile_pool(name="kv_keep", bufs=1))

        ident = const.tile([128, 128], F32)
        make_identity(nc, ident[:])
        ones_f = const.tile([SKVP, 2], F32)
        nc.gpsimd.memset(ones_f[:], 0.0)
        nc.gpsimd.memset(ones_f[0:SKV, :], 1.0)
        ones77 = const.tile([SKVP, 2], F32R)
        nc.vector.tensor_copy(ones77[:], ones_f[:])

        # Per-partition bias tiles: [:, m] is the bias for feature chunk m.
        bq_sb = const.tile([128, ME], F32)
        nc.sync.dma_start(bq_sb[:], bq.rearrange("(m p) -> p m", p=128))
        bk_sb = const.tile([128, ME], F32)
        nc.sync.dma_start(bk_sb[:], bk.rearrange("(m p) -> p m", p=128))

        # Free-axis biases, broadcast across partitions once.
        bo_row = const.tile([1, DE], F32)
        nc.sync.dma_start(bo_row[:], bo.rearrange("(o d) -> o d", o=1))
        bo_bc = const.tile([128, DE], F32)
        nc.gpsimd.partition_broadcast(bo_bc[:], bo_row[:])
        bv_row = const.tile([1, DE], F32)
        nc.sync.dma_start(bv_row[:], bv.rearrange("(o d) -> o d", o=1))
        bv_bc = const.tile([128, DE], F32)
        nc.gpsimd.partition_broadcast(bv_bc[:], bv_row[:])

        # Weights resident in SBUF: chunk k of Wq lives at [:, k*DE : (k+1)*DE].
        Wq_sb = wq_pool.tile([128, ME * DE], F32R)
        for k in range(ME):
            nc.sync.dma_start(Wq_sb[:, k * DE:(k + 1) * DE], Wq[k * 128:(k + 1) * 128, :])
        Wo_sb = wo_pool.tile([128, ME * DE], F32R)
        for k in range(ME):
            nc.sync.dma_start(Wo_sb[:, k * DE:(k + 1) * DE], Wo[k * 128:(k + 1) * 128, :])

        # Persistent K/V for attention.
        kT_sb = kv_keep.tile([128, H, SKVP], F32R)  # kT_sb[:, h, :] = K_h^T
        v_sb = kv_keep.tile([SKVP, DE], F32R)       # v (kv on partitions)

        # ---- Phase 1: k/v projections (runs once; pools released after) ----
        with tc.tile_pool(name="kv_tmp", bufs=1) as kvp, \
             tc.tile_pool(name="kv_psum", bufs=2, space="PSUM") as kvps:
            Wk_sb = kvp.tile([128, MC * DE], F32R)
            for c in range(MC):
                nc.sync.dma_start(Wk_sb[:, c * DE:(c + 1) * DE], Wk[c * 128:(c + 1) * 128, :])
            Wv_sb = kvp.tile([128, MC * DE], F32R)
            for c in range(MC):
                nc.sync.dma_start(Wv_sb[:, c * DE:(c + 1) * DE], Wv[c * 128:(c + 1) * 128, :])
            y_sb = kvp.tile([SKVP, DC], F32)
            nc.gpsimd.memset(y_sb[:], 0.0)
            nc.sync.dma_start(y_sb[0:SKV, :], y)

            # yT[:, c, :] = y[:, c*128:(c+1)*128].T
            yT = kvp.tile([128, MC, SKVP], F32R)
            for c in range(MC):
                tp_ps = kvps.tile([128, SKVP], F32, tag="tp")
                nc.tensor.transpose(
                    tp_ps[:],
                    y_sb[:, c * 128:(c + 1) * 128],
                    ident[0:SKVP, 0:SKVP],
                )
                nc.vector.tensor_copy(yT[:, c, :], tp_ps[:])

            # kT chunks: [128, 77] = (Wk chunk).T-contracted against yT.
            for m in range(ME):
                kps = kvps.tile([128, SKVP], F32, tag="kmm")
                for c in range(MC):
                    nc.tensor.matmul(
                        kps[:],
                        Wk_sb[:, c * DE + m * 128: c * DE + (m + 1) * 128],
                        yT[:, c, :],
                        start=(c == 0), stop=(c == MC - 1),
                    )
                nc.scalar.activation(kT_sb[:, m, :], kps[:], AF.Identity,
                                     bias=bk_sb[:, m:m + 1])
            # v: [77, 512] x 2
            for o in range(2):
                vps = kvps.tile([SKVP, 512], F32, tag="vmm")
                for c in range(MC):
                    nc.tensor.matmul(
                        vps[:],
                        yT[:, c, :],
                        Wv_sb[:, c * DE + o * 512: c * DE + (o + 1) * 512],
                        start=(c == 0), stop=(c == MC - 1),
                    )
                nc.vector.tensor_copy(v_sb[64:SKVP, o * 512:(o + 1) * 512],
                                      vps[64:SKVP, :])
                nc.vector.tensor_add(v_sb[0:SKV, o * 512:(o + 1) * 512], vps[0:SKV, :],
                                     bv_bc[0:SKV, o * 512:(o + 1) * 512])

        # ---- Phase 2: main loop over query blocks ----
        blkp = ctx.enter_context(tc.tile_pool(name="blk", bufs=1))
        psum = ctx.enter_context(tc.tile_pool(name="psum", bufs=1, space="PSUM"))

        for blk in range(nblk):
            q0 = blk * QB

            # Load x rows and transpose on PE into xT (features on partitions).
            xT = blkp.tile([128, ME, QB], F32R, tag="xT", bufs=1)
            for t in range(QB // 128):
                xr = blkp.tile([128, DE], F32, tag="xr", bufs=5)
                nc.sync.dma_start(xr[:], x[q0 + t * 128: q0 + (t + 1) * 128, :])
                for e in range(ME):
                    tp_ps = psum.tile([128, 128], F32, tag="tp", bufs=2)
                    nc.tensor.transpose(
                        tp_ps[:],
                        xr[:, e * 128:(e + 1) * 128],
                        ident[:],
                    )
                    nc.vector.tensor_copy(xT[:, e, t * 128:(t + 1) * 128], tp_ps[:])

            # Stage A: qT = Wq.T @ xT + bq
            qT = blkp.tile([128, ME, QB], F32R, tag="qT", bufs=1)
            for m in range(ME):
                qps = psum.tile([128, QB], F32, tag="mm", bufs=3)
                for k in range(ME):
                    nc.tensor.matmul(
                        qps[:],
                        Wq_sb[:, k * DE + m * 128: k * DE + (m + 1) * 128],
                        xT[:, k, :],
                        start=(k == 0), stop=(k == ME - 1),
                    )
                nc.scalar.activation(qT[:, m, :], qps[:], AF.Identity,
                                     bias=bq_sb[:, m:m + 1])

            # Attention per head.
            attnT = blkp.tile([128, H, QB], F32R, tag="attnT", bufs=2)
            for h in range(H):
                sps = psum.tile([SKVP, QB], F32, tag="small", bufs=3)
                nc.tensor.matmul(sps[:], kT_sb[:, h, :], qT[:, h, :],
                                 start=True, stop=True)
                pe_sb = blkp.tile([SKVP, QB], F32R, tag="probs", bufs=3)
                nc.scalar.activation(pe_sb[:], sps[:], AF.Exp, scale=SCALE)
                sums_ps = psum.tile([2, QB], F32, tag="small", bufs=3)
                nc.tensor.matmul(sums_ps[:], ones77[:], pe_sb[:],
                                 start=True, stop=True)
                recip = blkp.tile([1, QB], F32, tag="recip", bufs=3)
                nc.vector.reciprocal(recip[:], sums_ps[0:1, :])
                bcast = blkp.tile([128, QB], F32, tag="bcast", bufs=3)
                nc.gpsimd.partition_broadcast(bcast[:], recip[:])
                ups = psum.tile([128, QB], F32, tag="mm", bufs=3)
                nc.tensor.matmul(ups[:], v_sb[:, h * 128:(h + 1) * 128], pe_sb[:],
                                 start=True, stop=True)
                nc.vector.tensor_mul(attnT[:, h, :], ups[:], bcast[:])

            # Stage E: out = attnT.T @ Wo + bo   (queries back on partitions)
            for t in range(QB // 128):
                out_sb = blkp.tile([128, DE], F32, tag="out", bufs=3)
                for o in range(2):
                    eps = psum.tile([128, 512], F32, tag="mm", bufs=3)
                    for k in range(ME):
                        nc.tensor.matmul(
                            eps[:],
                            attnT[:, k, t * 128:(t + 1) * 128],
                            Wo_sb[:, k * DE + o * 512: k * DE + (o + 1) * 512],
                            start=(k == 0), stop=(k == ME - 1),
                        )
                    nc.vector.tensor_add(out_sb[:, o * 512:(o + 1) * 512], eps[:],
                                         bo_bc[:, o * 512:(o + 1) * 512])
                nc.sync.dma_start(out[q0 + t * 128: q0 + (t + 1) * 128, :], out_sb[:])

    nc.compile()
    return nc


_nc_cache = {}


def _get_nc(sq=SQ):
    if sq not in _nc_cache:
        _nc_cache[sq] = build_nc(sq)
    return _nc_cache[sq]


def kernel(**inputs):
    x = np.ascontiguousarray(np.asarray(inputs["x"], dtype=np.float32))
    y = np.ascontiguousarray(np.asarray(inputs["y"], dtype=np.float32))
    weights = {
        k: np.ascontiguousarray(np.asarray(inputs[k], dtype=np.float32))
        for k in ("Wq", "bq", "Wk", "bk", "Wv", "bv", "Wo", "bo")
    }

    nc = _get_nc()
    in_maps = [dict(x=x[b], y=y[b], **weights) for b in range(B)]
    res = bass_utils.run_bass_kernel_spmd(nc, in_maps, core_ids=list(range(NCORES)))
    return np.stack([r["out"] for r in res.results], axis=0)


# revision 12
# speedup vs baseline: 1.2557x; 1.2557x over previous
"""Trainium2 Bass kernel for nn_CrossAttention (B=8, SQ=4096, SKV=77,
D_EMBED=1024, D_CROSS=768, H=8, Dh=128), fp32 in/out.

Strategy: data-parallel over batch — one batch element per NeuronCore (8
cores).  Per core, everything is computed in "transposed" space (features on
partitions) so that no operand ever needs a transpose except the input x
itself (done on the PE via is_transpose matmuls) and softmax sums (done as a
ones-matmul over the partition axis):

  xT  = transpose(x_blk)                      [1024, QB]   (PE transpose)
  qT  = Wq.T @ xT + bq                        [1024, QB]   (features on part.)
  kT  = Wk.T @ yT + bk                        [1024, 77]
  v   = y @ Wv + bv                           [77, 1024]
  sT_h  = kT_h.T @ qT_h  (K=Dh on part.)      [77, QB] per head
  e_h   = exp(sT_h / sqrt(Dh))                [77, QB]
  sum_h = ones.T @ e_h                        [1, QB]
  uT_h  = v_h.T @ e_h    (K=77 on part.)      [128, QB]
  aT_h  = uT_h * broadcast(1/sum_h)           [128, QB]
  out   = aT.T @ Wo + bo                      [QB, 1024]  (queries on part.)

Matmuls run in float32r (full-rate fp32 PE mode); operand tensors/tiles are
declared float32r end-to-end (numpy sees plain float32 — same bytes).
"""

import math
import os
from contextlib import ExitStack

import numpy as np

os.environ.setdefault("JAX_PLATFORMS", "cpu")

import concourse.bass as bass  # noqa: E402
import concourse.tile as tile  # noqa: E402
from concourse import bacc, mybir  # noqa: E402
from concourse import bass_utils  # noqa: E402
from concourse.masks import make_identity  # noqa: E402

F32 = mybir.dt.float32
F32R = mybir.dt.float32r
AF = mybir.ActivationFunctionType

# Persist compiled executables across processes: the bass_exec HLO is
# deterministic for a given build, so the (slow) neuronxcc compile is hit
# only once per machine.
try:
    import jax

    jax.config.update("jax_compilation_cache_dir", "/tmp/jax_bass_cache")
    jax.config.update("jax_persistent_cache_min_compile_time_secs", 0)
    jax.config.update("jax_persistent_cache_min_entry_size_bytes", 0)
except Exception:
    pass

# Problem constants (hardcoded per contract).
B = 8
SQ = 4096
SKV = 77
DE = 1024
DC = 768
H = 8
DH = 128
NCORES = 8

QB = 512          # queries per block
ME = DE // 128    # 8 feature chunks of the embed dim
MC = DC // 128    # 6 chunks of the cross dim
SCALE = 1.0 / math.sqrt(DH)
SKVP = 80          # SKV padded to even size (fp32r dst patterns must be even)


def build_nc(sq=SQ):
    """Build the per-core Bass program (identical on all cores)."""
    nblk = sq // QB
    nc = bacc.Bacc(
        "TRN2",
        target_bir_lowering=False,
        debug=False,
        enable_asserts=False,
        num_devices=NCORES,
    )
    x = nc.dram_tensor("x", [sq, DE], F32, kind="ExternalInput").ap()
    y = nc.dram_tensor("y", [SKV, DC], F32, kind="ExternalInput").ap()
    Wq = nc.dram_tensor("Wq", [DE, DE], F32R, kind="ExternalInput").ap()
    bq = nc.dram_tensor("bq", [DE], F32, kind="ExternalInput").ap()
    Wk = nc.dram_tensor("Wk", [DC, DE], F32R, kind="ExternalInput").ap()
    bk = nc.dram_tensor("bk", [DE], F32, kind="ExternalInput").ap()
    Wv = nc.dram_tensor("Wv", [DC, DE], F32R, kind="ExternalInput").ap()
    bv = nc.dram_tensor("bv", [DE], F32, kind="ExternalInput").ap()
    Wo = nc.dram_tensor("Wo", [DE, DE], F32R, kind="ExternalInput").ap()
    bo = nc.dram_tensor("bo", [DE], F32, kind="ExternalInput").ap()
    out = nc.dram_tensor("out", [sq, DE], F32, kind="ExternalOutput").ap()

    with tile.TileContext(nc) as tc, ExitStack() as ctx:
        const = ctx.enter_context(tc.tile_pool(name="const", bufs=1))
        wq_pool = ctx.enter_context(tc.tile_pool(name="wq", bufs=1))
        wo_pool = ctx.enter_context(tc.tile_pool(name="wo", bufs=1))
        kv_keep = ctx.enter_context(tc.tile_pool(name="kv_keep", bufs=1))

        ident = const.tile([128, 128], F32)
        make_identity(nc, ident[:])
        ones_f = const.tile([SKVP, 2], F32)
        nc.gpsimd.memset(ones_f[:], 0.0)
        nc.gpsimd.memset(ones_f[0:SKV, :], 1.0)
        ones77 = const.tile([SKVP, 2], F32R)
        nc.vector.tensor_copy(ones77[:], ones_f[:])

        # Per-partition bias tiles: [:, m] is the bias for feature chunk m.
        bq_sb = const.tile([128, ME], F32)
        nc.sync.dma_start(bq_sb[:], bq.rearrange("(m p) -> p m", p=128))
        bk_sb = const.tile([128, ME], F32)
        nc.sync.dma_start(bk_sb[:], bk.rearrange("(m p) -> p m", p=128))

        # Free-axis biases, broadcast across partitions once.
        bo_row = const.tile([1, DE], F32)
        nc.sync.dma_start(bo_row[:], bo.rearrange("(o d) -> o d", o=1))
        bo_bc = const.tile([128, DE], F32)
        nc.gpsimd.partition_broadcast(bo_bc[:], bo_row[:])
        bv_row = const.tile([1, DE], F32)
        nc.sync.dma_start(bv_row[:], bv.rearrange("(o d) -> o d", o=1))
        bv_bc = const.tile([128, DE], F32)
        nc.gpsimd.partition_broadcast(bv_bc[:], bv_row[:])

        # Weights resident in SBUF: chunk k of Wq lives at [:, k*DE : (k+1)*DE].
        Wq_sb = wq_pool.tile([128, ME * DE], F32R)
        for k in range(ME):
            nc.sync.dma_start(Wq_sb[:, k * DE:(k + 1) * DE], Wq[k * 128:(k + 1) * 128, :])
        Wo_sb = wo_pool.tile([128, ME * DE], F32R)
        for k in range(ME):
            nc.sync.dma_start(Wo_sb[:, k * DE:(k + 1) * DE], Wo[k * 128:(k + 1) * 128, :])

        # Persistent K/V for attention.
        kT_sb = kv_keep.tile([128, H, SKVP], F32R)  # kT_sb[:, h, :] = K_h^T
        v_sb = kv_keep.tile([SKVP, DE], F32R)       # v (kv on partitions)

        # ---- Phase 1: k/v projections (runs once; pools released after) ----
        with tc.tile_pool(name="kv_tmp", bufs=1) as kvp, \
             tc.tile_pool(name="kv_psum", bufs=2, space="PSUM") as kvps:
            Wk_sb = kvp.tile([128, MC * DE], F32R)
            for c in range(MC):
                nc.sync.dma_start(Wk_sb[:, c * DE:(c + 1) * DE], Wk[c * 128:(c + 1) * 128, :])
            Wv_sb = kvp.tile([128, MC * DE], F32R)
            for c in range(MC):
                nc.sync.dma_start(Wv_sb[:, c * DE:(c + 1) * DE], Wv[c * 128:(c + 1) * 128, :])
            y_sb = kvp.tile([SKVP, DC], F32)
            nc.gpsimd.memset(y_sb[:], 0.0)
            nc.sync.dma_start(y_sb[0:SKV, :], y)

            # yT[:, c, :] = y[:, c*128:(c+1)*128].T
            yT = kvp.tile([128, MC, SKVP], F32R)
            for c in range(MC):
                tp_ps = kvps.tile([128, SKVP], F32, tag="tp")
                nc.tensor.transpose(
                    tp_ps[:],
                    y_sb[:, c * 128:(c + 1) * 128],
                    ident[0:SKVP, 0:SKVP],
                )
                nc.vector.tensor_copy(yT[:, c, :], tp_ps[:])

            # kT chunks: [128, 77] = (Wk chunk).T-contracted against yT.
            for m in range(ME):
                kps = kvps.tile([128, SKVP], F32, tag="kmm")
                for c in range(MC):
                    nc.tensor.matmul(
                        kps[:],
                        Wk_sb[:, c * DE + m * 128: c * DE + (m + 1) * 128],
                        yT[:, c, :],
                        start=(c == 0), stop=(c == MC - 1),
                    )
                nc.scalar.activation(kT_sb[:, m, :], kps[:], AF.Identity,
                                     bias=bk_sb[:, m:m + 1])
            # v: [77, 512] x 2
            for o in range(2):
                vps = kvps.tile([SKVP, 512], F32, tag="vmm")
                for c in range(MC):
                    nc.tensor.matmul(
                        vps[:],
                        yT[:, c, :],
                        Wv_sb[:, c * DE + o * 512: c * DE + (o + 1) * 512],
                        start=(c == 0), stop=(c == MC - 1),
                    )
                nc.vector.tensor_copy(v_sb[64:SKVP, o * 512:(o + 1) * 512],
                                      vps[64:SKVP, :])
                nc.vector.tensor_add(v_sb[0:SKV, o * 512:(o + 1) * 512], vps[0:SKV, :],
                                     bv_bc[0:SKV, o * 512:(o + 1) * 512])

        # ---- Phase 2: main loop over query blocks ----
        blkp = ctx.enter_context(tc.tile_pool(name="blk", bufs=1))
        psum = ctx.enter_context(tc.tile_pool(name="psum", bufs=1, space="PSUM"))

        def emit_load_and_stage_a(blk):
            q0 = blk * QB
            # Load x rows and transpose on PE into xT (features on partitions).
            xT = blkp.tile([128, ME, QB], F32R, tag="xT", bufs=1, name=f"xT{blk}")
            for t in range(QB // 128):
                xr = blkp.tile([128, DE], F32, tag="xr", bufs=5, name=f"xr{blk}_{t}")
                nc.sync.dma_start(xr[:], x[q0 + t * 128: q0 + (t + 1) * 128, :])
                for e in range(ME):
                    tp_ps = psum.tile([128, 128], F32, tag="tp", bufs=2,
                                      name=f"tp{blk}_{t}_{e}")
                    nc.tensor.transpose(
                        tp_ps[:],
                        xr[:, e * 128:(e + 1) * 128],
                        ident[:],
                    )
                    nc.vector.tensor_copy(xT[:, e, t * 128:(t + 1) * 128], tp_ps[:])

            # Stage A: qT = Wq.T @ xT + bq
            qT = blkp.tile([128, ME, QB], F32R, tag="qT", bufs=1, name=f"qT{blk}")
            for m in range(ME):
                qps = psum.tile([128, QB], F32, tag="mm", bufs=3,
                                name=f"qps{blk}_{m}")
                for k in range(ME):
                    nc.tensor.matmul(
                        qps[:],
                        Wq_sb[:, k * DE + m * 128: k * DE + (m + 1) * 128],
                        xT[:, k, :],
                        start=(k == 0), stop=(k == ME - 1),
                    )
                nc.scalar.activation(qT[:, m, :], qps[:], AF.Identity,
                                     bias=bq_sb[:, m:m + 1])
            return qT

        def emit_attention(blk, qT):
            attnT = blkp.tile([128, H, QB], F32R, tag="attnT", bufs=2,
                              name=f"attnT{blk}")
            for h in range(H):
                sps = psum.tile([SKVP, QB], F32, tag="small", bufs=3,
                                name=f"sps{blk}_{h}")
                nc.tensor.matmul(sps[:], kT_sb[:, h, :], qT[:, h, :],
                                 start=True, stop=True)
                pe_sb = blkp.tile([SKVP, QB], F32R, tag="probs", bufs=3,
                                  name=f"pe{blk}_{h}")
                nc.scalar.activation(pe_sb[:], sps[:], AF.Exp, scale=SCALE)
                sums_ps = psum.tile([2, QB], F32, tag="small", bufs=3,
                                    name=f"sums{blk}_{h}")
                nc.tensor.matmul(sums_ps[:], ones77[:], pe_sb[:],
                                 start=True, stop=True)
                recip = blkp.tile([1, QB], F32, tag="recip", bufs=3,
                                  name=f"recip{blk}_{h}")
                nc.vector.reciprocal_approx_fast(recip[:], sums_ps[0:1, :])
                bcast = blkp.tile([128, QB], F32, tag="bcast", bufs=3,
                                  name=f"bcast{blk}_{h}")
                nc.gpsimd.partition_broadcast(bcast[:], recip[:])
                ups = psum.tile([128, QB], F32, tag="mm", bufs=3,
                                name=f"ups{blk}_{h}")
                nc.tensor.matmul(ups[:], v_sb[:, h * 128:(h + 1) * 128], pe_sb[:],
                                 start=True, stop=True)
                nc.vector.tensor_mul(attnT[:, h, :], ups[:], bcast[:])
            return attnT

        def emit_stage_e(blk, attnT):
            q0 = blk * QB
            for t in range(QB // 128):
                out_sb = blkp.tile([128, DE], F32, tag="out", bufs=3,
                                   name=f"out{blk}_{t}")
                for o in range(2):
                    eps = psum.tile([128, 512], F32, tag="mm", bufs=3,
                                    name=f"eps{blk}_{t}_{o}")
                    for k in range(ME):
                        nc.tensor.matmul(
                            eps[:],
                            attnT[:, k, t * 128:(t + 1) * 128],
                            Wo_sb[:, k * DE + o * 512: k * DE + (o + 1) * 512],
                            start=(k == 0), stop=(k == ME - 1),
                        )
                    nc.vector.tensor_add(out_sb[:, o * 512:(o + 1) * 512], eps[:],
                                         bo_bc[:, o * 512:(o + 1) * 512])
                nc.sync.dma_start(out[q0 + t * 128: q0 + (t + 1) * 128, :], out_sb[:])

        # Software pipeline: stage E of block i-1 is emitted between stage A
        # and attention of block i, so the PE has dense matmul work while the
        # previous block's attention ACT/DVE tail drains, keeping HAM warm.
        prev = None
        for blk in range(nblk):
            qT = emit_load_and_stage_a(blk)
            if prev is not None:
                emit_stage_e(prev[0], prev[1])
            attnT = emit_attention(blk, qT)
            prev = (blk, attnT)
        emit_stage_e(prev[0], prev[1])

    nc.compile()
    return nc


_nc_cache = {}


def _get_nc(sq=SQ):
    if sq not in _nc_cache:
        _nc_cache[sq] = build_nc(sq)
    return _nc_cache[sq]


def kernel(**inputs):
    x = np.ascontiguousarray(np.asarray(inputs["x"], dtype=np.float32))
    y = np.ascontiguousarray(np.asarray(inputs["y"], dtype=np.float32))
    weights = {
        k: np.ascontiguousarray(np.asarray(inputs[k], dtype=np.float32))
        for k in ("Wq", "bq", "Wk", "bk", "Wv", "bv", "Wo", "bo")
    }

    nc = _get_nc()
    in_maps = [dict(x=x[b], y=y[b], **weights) for b in range(B)]
    res = bass_utils.run_bass_kernel_spmd(nc, in_maps, core_ids=list(range(NCORES)))
    return np.stack([r["out"] for r in res.results], axis=0)


# revision 13
# speedup vs baseline: 1.3095x; 1.0428x over previous
"""Trainium2 Bass kernel for nn_CrossAttention (B=8, SQ=4096, SKV=77,
D_EMBED=1024, D_CROSS=768, H=8, Dh=128), fp32 in/out.

Strategy: data-parallel over batch — one batch element per NeuronCore (8
cores).  Per core, everything is computed in "transposed" space (features on
partitions) so that no operand ever needs a transpose except the input x
itself (done on the PE via is_transpose matmuls) and softmax sums (done as a
ones-matmul over the partition axis):

  xT  = transpose(x_blk)                      [1024, QB]   (PE transpose)
  qT  = Wq.T @ xT + bq                        [1024, QB]   (features on part.)
  kT  = Wk.T @ yT + bk                        [1024, 77]
  v   = y @ Wv + bv                           [77, 1024]
  sT_h  = kT_h.T @ qT_h  (K=Dh on part.)      [77, QB] per head
  e_h   = exp(sT_h / sqrt(Dh))                [77, QB]
  sum_h = ones.T @ e_h                        [1, QB]
  uT_h  = v_h.T @ e_h    (K=77 on part.)      [128, QB]
  aT_h  = uT_h * broadcast(1/sum_h)           [128, QB]
  out   = aT.T @ Wo + bo                      [QB, 1024]  (queries on part.)

Matmuls run in float32r (full-rate fp32 PE mode); operand tensors/tiles are
declared float32r end-to-end (numpy sees plain float32 — same bytes).
"""

import math
import os
from contextlib import ExitStack

import numpy as np

os.environ.setdefault("JAX_PLATFORMS", "cpu")

import concourse.bass as bass  # noqa: E402
import concourse.tile as tile  # noqa: E402
from concourse import bacc, mybir  # noqa: E402
from concourse import bass_utils  # noqa: E402
from concourse.masks import make_identity  # noqa: E402

F32 = mybir.dt.float32
F32R = mybir.dt.float32r
AF = mybir.ActivationFunctionType

# Persist compiled executables across processes: the bass_exec HLO is
# deterministic for a given build, so the (slow) neuronxcc compile is hit
# only once per machine.
try:
    import jax

    jax.config.update("jax_compilation_cache_dir", "/tmp/jax_bass_cache")
    jax.config.update("jax_persistent_cache_min_compile_time_secs", 0)
    jax.config.update("jax_persistent_cache_min_entry_size_bytes", 0)
except Exception:
    pass

# Problem constants (hardcoded per contract).
B = 8
SQ = 4096
SKV = 77
DE = 1024
DC = 768
H = 8
DH = 128
NCORES = 8

QB = 512          # queries per block
ME = DE // 128    # 8 feature chunks of the embed dim
MC = DC // 128    # 6 chunks of the cross dim
SCALE = 1.0 / math.sqrt(DH)
SKVP = 80          # SKV padded to even size (fp32r dst patterns must be even)


def build_nc(sq=SQ):
    """Build the per-core Bass program (identical on all cores)."""
    nblk = sq // QB
    nc = bacc.Bacc(
        "TRN2",
        target_bir_lowering=False,
        debug=False,
        enable_asserts=False,
        num_devices=NCORES,
    )
    x = nc.dram_tensor("x", [sq, DE], F32R, kind="ExternalInput").ap()
    y = nc.dram_tensor("y", [SKV, DC], F32, kind="ExternalInput").ap()
    Wq = nc.dram_tensor("Wq", [DE, DE], F32R, kind="ExternalInput").ap()
    bq = nc.dram_tensor("bq", [DE], F32, kind="ExternalInput").ap()
    Wk = nc.dram_tensor("Wk", [DC, DE], F32R, kind="ExternalInput").ap()
    bk = nc.dram_tensor("bk", [DE], F32, kind="ExternalInput").ap()
    Wv = nc.dram_tensor("Wv", [DC, DE], F32R, kind="ExternalInput").ap()
    bv = nc.dram_tensor("bv", [DE], F32, kind="ExternalInput").ap()
    Wo = nc.dram_tensor("Wo", [DE, DE], F32R, kind="ExternalInput").ap()
    bo = nc.dram_tensor("bo", [DE], F32, kind="ExternalInput").ap()
    out = nc.dram_tensor("out", [sq, DE], F32, kind="ExternalOutput").ap()

    with tile.TileContext(nc) as tc, ExitStack() as ctx:
        const = ctx.enter_context(tc.tile_pool(name="const", bufs=1))
        wq_pool = ctx.enter_context(tc.tile_pool(name="wq", bufs=1))
        wo_pool = ctx.enter_context(tc.tile_pool(name="wo", bufs=1))
        kv_keep = ctx.enter_context(tc.tile_pool(name="kv_keep", bufs=1))

        ident = const.tile([128, 128], F32)
        make_identity(nc, ident[:])
        ident_r = const.tile([128, 128], F32R)
        nc.vector.tensor_copy(ident_r[:], ident[:])
        ones_f = const.tile([SKVP, 2], F32)
        nc.gpsimd.memset(ones_f[:], 0.0)
        nc.gpsimd.memset(ones_f[0:SKV, :], 1.0)
        ones77 = const.tile([SKVP, 2], F32R)
        nc.vector.tensor_copy(ones77[:], ones_f[:])

        # Per-partition bias tiles: [:, m] is the bias for feature chunk m.
        bq_sb = const.tile([128, ME], F32)
        nc.sync.dma_start(bq_sb[:], bq.rearrange("(m p) -> p m", p=128))
        bk_sb = const.tile([128, ME], F32)
        nc.sync.dma_start(bk_sb[:], bk.rearrange("(m p) -> p m", p=128))

        # Free-axis biases, broadcast across partitions once.
        bo_row = const.tile([1, DE], F32)
        nc.sync.dma_start(bo_row[:], bo.rearrange("(o d) -> o d", o=1))
        bo_bc = const.tile([128, DE], F32)
        nc.gpsimd.partition_broadcast(bo_bc[:], bo_row[:])
        bv_row = const.tile([1, DE], F32)
        nc.sync.dma_start(bv_row[:], bv.rearrange("(o d) -> o d", o=1))
        bv_bc = const.tile([128, DE], F32)
        nc.gpsimd.partition_broadcast(bv_bc[:], bv_row[:])

        # Weights resident in SBUF: chunk k of Wq lives at [:, k*DE : (k+1)*DE].
        Wq_sb = wq_pool.tile([128, ME * DE], F32R)
        for k in range(ME):
            nc.sync.dma_start(Wq_sb[:, k * DE:(k + 1) * DE], Wq[k * 128:(k + 1) * 128, :])
        Wo_sb = wo_pool.tile([128, ME * DE], F32R)
        for k in range(ME):
            nc.sync.dma_start(Wo_sb[:, k * DE:(k + 1) * DE], Wo[k * 128:(k + 1) * 128, :])

        # Persistent K/V for attention.
        kT_sb = kv_keep.tile([128, H, SKVP], F32R)  # kT_sb[:, h, :] = K_h^T
        v_sb = kv_keep.tile([SKVP, DE], F32R)       # v (kv on partitions)

        # ---- Phase 1: k/v projections (runs once; pools released after) ----
        with tc.tile_pool(name="kv_tmp", bufs=1) as kvp, \
             tc.tile_pool(name="kv_psum", bufs=2, space="PSUM") as kvps:
            Wk_sb = kvp.tile([128, MC * DE], F32R)
            for c in range(MC):
                nc.sync.dma_start(Wk_sb[:, c * DE:(c + 1) * DE], Wk[c * 128:(c + 1) * 128, :])
            Wv_sb = kvp.tile([128, MC * DE], F32R)
            for c in range(MC):
                nc.sync.dma_start(Wv_sb[:, c * DE:(c + 1) * DE], Wv[c * 128:(c + 1) * 128, :])
            y_sb = kvp.tile([SKVP, DC], F32)
            nc.gpsimd.memset(y_sb[:], 0.0)
            nc.sync.dma_start(y_sb[0:SKV, :], y)

            # yT[:, c, :] = y[:, c*128:(c+1)*128].T
            yT = kvp.tile([128, MC, SKVP], F32R)
            for c in range(MC):
                tp_ps = kvps.tile([128, SKVP], F32, tag="tp")
                nc.tensor.transpose(
                    tp_ps[:],
                    y_sb[:, c * 128:(c + 1) * 128],
                    ident[0:SKVP, 0:SKVP],
                )
                nc.vector.tensor_copy(yT[:, c, :], tp_ps[:])

            # kT chunks: [128, 77] = (Wk chunk).T-contracted against yT.
            for m in range(ME):
                kps = kvps.tile([128, SKVP], F32, tag="kmm")
                for c in range(MC):
                    nc.tensor.matmul(
                        kps[:],
                        Wk_sb[:, c * DE + m * 128: c * DE + (m + 1) * 128],
                        yT[:, c, :],
                        start=(c == 0), stop=(c == MC - 1),
                    )
                nc.scalar.activation(kT_sb[:, m, :], kps[:], AF.Identity,
                                     bias=bk_sb[:, m:m + 1])
            # v: [77, 512] x 2
            for o in range(2):
                vps = kvps.tile([SKVP, 512], F32, tag="vmm")
                for c in range(MC):
                    nc.tensor.matmul(
                        vps[:],
                        yT[:, c, :],
                        Wv_sb[:, c * DE + o * 512: c * DE + (o + 1) * 512],
                        start=(c == 0), stop=(c == MC - 1),
                    )
                nc.vector.tensor_copy(v_sb[64:SKVP, o * 512:(o + 1) * 512],
                                      vps[64:SKVP, :])
                nc.vector.tensor_add(v_sb[0:SKV, o * 512:(o + 1) * 512], vps[0:SKV, :],
                                     bv_bc[0:SKV, o * 512:(o + 1) * 512])

        # ---- Phase 2: main loop over query blocks ----
        blkp = ctx.enter_context(tc.tile_pool(name="blk", bufs=1))
        psum = ctx.enter_context(tc.tile_pool(name="psum", bufs=1, space="PSUM"))

        def emit_load_and_stage_a(blk):
            q0 = blk * QB
            # Load x rows and transpose on PE into xT (features on partitions).
            xT = blkp.tile([128, ME, QB], F32R, tag="xT", bufs=1, name=f"xT{blk}")
            for t in range(QB // 128):
                xr = blkp.tile([128, DE], F32R, tag="xr", bufs=5, name=f"xr{blk}_{t}")
                nc.sync.dma_start(xr[:], x[q0 + t * 128: q0 + (t + 1) * 128, :])
                for e in range(ME):
                    tp_ps = psum.tile([128, 128], F32, tag="tp", bufs=2,
                                      name=f"tp{blk}_{t}_{e}")
                    nc.tensor.transpose(
                        tp_ps[:].bitcast(F32R),
                        xr[:, e * 128:(e + 1) * 128],
                        ident_r[:],
                    )
                    nc.vector.tensor_copy(xT[:, e, t * 128:(t + 1) * 128],
                                          tp_ps[:].bitcast(F32R))

            # Stage A: qT = Wq.T @ xT + bq
            qT = blkp.tile([128, ME, QB], F32R, tag="qT", bufs=1, name=f"qT{blk}")
            for m in range(ME):
                qps = psum.tile([128, QB], F32, tag="mm", bufs=3,
                                name=f"qps{blk}_{m}")
                for k in range(ME):
                    nc.tensor.matmul(
                        qps[:],
                        Wq_sb[:, k * DE + m * 128: k * DE + (m + 1) * 128],
                        xT[:, k, :],
                        start=(k == 0), stop=(k == ME - 1),
                    )
                nc.scalar.activation(qT[:, m, :], qps[:], AF.Identity,
                                     bias=bq_sb[:, m:m + 1])
            return qT

        def emit_scores(blk, qT, h):
            sps = psum.tile([SKVP, QB], F32, tag="small", bufs=3,
                            name=f"sps{blk}_{h}")
            nc.tensor.matmul(sps[:], kT_sb[:, h, :], qT[:, h, :],
                             start=True, stop=True)
            pe_sb = blkp.tile([SKVP, QB], F32R, tag="probs", bufs=3,
                              name=f"pe{blk}_{h}")
            nc.scalar.activation(pe_sb[:], sps[:], AF.Exp, scale=SCALE)
            return pe_sb

        def emit_norm(blk, attnT, h, pe_sb):
            sums_ps = psum.tile([2, QB], F32, tag="small", bufs=3,
                                name=f"sums{blk}_{h}")
            nc.tensor.matmul(sums_ps[:], ones77[:], pe_sb[:],
                             start=True, stop=True)
            recip = blkp.tile([1, QB], F32, tag="recip", bufs=3,
                              name=f"recip{blk}_{h}")
            nc.vector.reciprocal_approx_fast(recip[:], sums_ps[0:1, :])
            bcast = blkp.tile([128, QB], F32, tag="bcast", bufs=3,
                              name=f"bcast{blk}_{h}")
            nc.gpsimd.partition_broadcast(bcast[:], recip[:])
            ups = psum.tile([128, QB], F32, tag="mm", bufs=3,
                            name=f"ups{blk}_{h}")
            nc.tensor.matmul(ups[:], v_sb[:, h * 128:(h + 1) * 128], pe_sb[:],
                             start=True, stop=True)
            nc.vector.tensor_mul(attnT[:, h, :], ups[:], bcast[:])

        def emit_stage_e_group(blk, attnT, out_tiles, c):
            q0 = blk * QB
            t, o = c // 2, c % 2
            if o == 0:
                out_tiles[t] = blkp.tile([128, DE], F32, tag="out", bufs=3,
                                         name=f"out{blk}_{t}")
            out_sb = out_tiles[t]
            eps = psum.tile([128, 512], F32, tag="mm", bufs=3,
                            name=f"eps{blk}_{t}_{o}")
            for k in range(ME):
                nc.tensor.matmul(
                    eps[:],
                    attnT[:, k, t * 128:(t + 1) * 128],
                    Wo_sb[:, k * DE + o * 512: k * DE + (o + 1) * 512],
                    start=(k == 0), stop=(k == ME - 1),
                )
            nc.vector.tensor_add(out_sb[:, o * 512:(o + 1) * 512], eps[:],
                                 bo_bc[:, o * 512:(o + 1) * 512])
            if o == 1:
                nc.sync.dma_start(out[q0 + t * 128: q0 + (t + 1) * 128, :],
                                  out_sb[:])

        # Software pipeline with fine interleave: the 8 stage-E psum groups of
        # block i-1 are emitted between the attention matmuls of block i, so
        # the PE always has dense work while exp/normalize chains drain on
        # ACT/DVE, keeping HAM at full clock.
        LAG = 2
        prev = None
        for blk in range(nblk):
            qT = emit_load_and_stage_a(blk)
            attnT = blkp.tile([128, H, QB], F32R, tag="attnT", bufs=2,
                              name=f"attnT{blk}")
            out_tiles = {}
            pe_tiles = {}
            for c in range(H):
                if prev is not None:
                    emit_stage_e_group(prev[0], prev[1], out_tiles, c)
                pe_tiles[c] = emit_scores(blk, qT, c)
                if c >= LAG:
                    emit_norm(blk, attnT, c - LAG, pe_tiles.pop(c - LAG))
            for h in range(H - LAG, H):
                emit_norm(blk, attnT, h, pe_tiles.pop(h))
            prev = (blk, attnT)
        out_tiles = {}
        for c in range(H):
            emit_stage_e_group(prev[0], prev[1], out_tiles, c)

    nc.compile()
    return nc


_nc_cache = {}


def _get_nc(sq=SQ):
    if sq not in _nc_cache:
        _nc_cache[sq] = build_nc(sq)
    return _nc_cache[sq]


def kernel(**inputs):
    x = np.ascontiguousarray(np.asarray(inputs["x"], dtype=np.float32))
    y = np.ascontiguousarray(np.asarray(inputs["y"], dtype=np.float32))
    weights = {
        k: np.ascontiguousarray(np.asarray(inputs[k], dtype=np.float32))
        for k in ("Wq", "bq", "Wk", "bk", "Wv", "bv", "Wo", "bo")
    }

    nc = _get_nc()
    in_maps = [dict(x=x[b], y=y[b], **weights) for b in range(B)]
    res = bass_utils.run_bass_kernel_spmd(nc, in_maps, core_ids=list(range(NCORES)))
    return np.stack([r["out"] for r in res.results], axis=0)
